# revision 56
# baseline (speedup 1.0000x reference)
"""Trainium2 Bass kernel for nn_Attention (B=4, S=2048, D=2048, H=16, KV=4, HD=128).

Sharding (8 cores): data-parallel over batch (4) x tensor-parallel over
KV-head-group halves (2). Core c handles batch b=c//2 and q-heads
[8*(c%2), 8*(c%2)+8) == kv groups {2*(c%2), 2*(c%2)+1}. Each core produces a
partial output (its heads' contribution through wo); the host sums the two
partials per batch.

v2: Q/K/V projections run as fp8e4 DoubleRow matmuls (0.5 cycles/row, two
128-deep contraction subtiles per instruction) with an error-corrected hi/lo
split of both x and the weights (3 of 4 cross terms; ~2^-8 effective
precision). Weights are pre-scaled x32 on the host so the lo residual clears
e4m3's subnormal floor; the descale folds into cos/sin (q,k) and a 32.0
ones-column in the softmax denominator (v). Attention (scores, exp, AV) and
the output projection run in bf16. Softmax denominators come from
probs-stationary x ones-moving matmuls (out [128q,1], ~1 cycle each instead
of 512); the reciprocal row reaches [dmodel, q] broadcast layout via a tiny
transpose + selector matmuls. Normalization is fused into the PSUM->SBUF
copy of AV. Out-projection chunks are interleaved into the next
q-superblock's attention so the single PSUM bank never stalls the PE.
"""
import numpy as np

B, S, D = 4, 2048, 2048
H, KV, HD = 16, 4, 128
NREP = H // KV
SCALE = float(HD) ** -0.5
WS = 32.0                  # host weight pre-scale (power of 2)

SB = S // 128          # 16 s-blocks
KT = D // 128          # 16 contraction tiles for projections
QSB = S // 512         # 4 q-superblocks
HPC = 8                # q heads per core
GPC = 2                # kv groups per core

_compiled = {}


def _build(causal: bool):
    import concourse.bass as bass  # noqa: F401
    import concourse.tile as tile
    from concourse import bacc, mybir
    from concourse.masks import make_identity

    f32 = mybir.dt.float32
    bf16 = mybir.dt.bfloat16
    f8 = mybir.dt.float8e4
    DR = mybir.MatmulPerfMode.DoubleRow
    AF = mybir.ActivationFunctionType
    ALU = mybir.AluOpType

    nc = bacc.Bacc("TRN2")

    # x hi/lo: [0]=hi, [1]=lo.  weights hi/lo: [0]=LO, [1]=HI (two-major layout)
    xT8 = nc.dram_tensor("xT8", [2, D, S], f8, kind="ExternalInput")
    wq8 = nc.dram_tensor("wq8", [2, D, HPC * HD], f8, kind="ExternalInput")
    wkv8 = nc.dram_tensor("wkv8", [2, D, 2 * GPC * HD], f8, kind="ExternalInput")
    wo8 = nc.dram_tensor("wo8", [2, HPC * HD, D], f8, kind="ExternalInput")
    cosS = nc.dram_tensor("cosS", [128, SB, 64], f32, kind="ExternalInput")
    sinS = nc.dram_tensor("sinS", [128, SB, 64], f32, kind="ExternalInput")
    mtile = nc.dram_tensor("mtile", [128, 128], f32, kind="ExternalInput")
    ones32 = nc.dram_tensor("ones32", [128, 1], bf16, kind="ExternalInput")
    selS = nc.dram_tensor("selS", [4, 4 * 128], bf16, kind="ExternalInput")
    outT = nc.dram_tensor("outT", [D, S], bf16, kind="ExternalOutput")

    xT4 = xT8.rearrange("two (kt p) s -> p two kt s", p=128)
    wq4 = wq8.rearrange("two (kt p) e -> p two kt e", p=128)
    wkv4 = wkv8.rearrange("two (kt p) e -> p two kt e", p=128)
    wo4 = wo8.rearrange("two (h p) d -> p two h d", p=128)

    with tile.TileContext(nc) as tc:
        with tc.tile_pool(name="persist", bufs=1) as persist:
            qT = [persist.tile([128, S], bf16, tag=f"qT{h}", name=f"qT{h}") for h in range(HPC)]
            kT = [persist.tile([128, S], bf16, tag=f"kTg{g}", name=f"kTg{g}") for g in range(GPC)]
            vsb = [persist.tile([128, SB, 128], bf16, tag=f"v{g}", name=f"v{g}") for g in range(GPC)]
            wos = persist.tile([128, 2, HPC, D], f8, tag="wos")
            msk = persist.tile([128, 128], f32, tag="msk")
            nc.sync.dma_start(out=msk, in_=mtile[:, :])
            onec = persist.tile([128, 1], bf16, tag="onec")
            nc.sync.dma_start(out=onec, in_=ones32[:, :])
            selt = persist.tile([4, 4, 128], bf16, tag="selt")
            sel3 = selS.rearrange("k (m d) -> k m d", d=128)
            nc.sync.dma_start(out=selt, in_=sel3[:, :, :])
            ident = persist.tile([128, 128], bf16, tag="ident")
            ident_f = persist.tile([128, 128], f32, tag="identf")
            make_identity(nc, ident_f)
            nc.vector.tensor_copy(out=ident, in_=ident_f)

            cos_t = persist.tile([128, SB, 64], f32, tag="cos")
            sin_t = persist.tile([128, SB, 64], f32, tag="sin")
            nc.sync.dma_start(out=cos_t, in_=cosS[:, :, :])
            nc.sync.dma_start(out=sin_t, in_=sinS[:, :, :])

            # ------------ Stage 1: projections + RoPE + transposes ----------

            # Single merged pass: per s-block compute all 12 projections
            # (8 q | 2 k | 2 v) from one x load. ps columns: q 0:1024,
            # kv 1024:1536 (K heads first). 10 heads get RoPE.
            EW = (HPC + 2 * GPC) * HD  # 1536
            NR = HPC + GPC             # 10 rope heads
            with tc.tile_pool(name="w1", bufs=1) as wpool, \
                 tc.tile_pool(name="xs1", bufs=2) as xpool, \
                 tc.tile_pool(name="rs1", bufs=2) as rpool, \
                 tc.tile_pool(name="pq1", bufs=2, space="PSUM") as pqp, \
                 tc.tile_pool(name="pt1", bufs=2, space="PSUM") as ptp:
                wtq = wpool.tile([128, 2, KT, HPC * HD], f8, tag="wtq")
                wtk = wpool.tile([128, 2, KT, 2 * GPC * HD], f8, tag="wtk")
                # prefetch sb0's x before the weights so compute can stream
                xs0 = xpool.tile([128, 2, KT, 128], f8, tag="xs")
                for v in range(2):
                    nc.sync.dma_start(out=xs0[:, v, 0:8, :],
                                      in_=xT4[:, v, 0:8, 0:128])
                    nc.sync.dma_start(out=xs0[:, v, 8:16, :],
                                      in_=xT4[:, v, 8:16, 0:128])
                # weights interleaved by kt-pair: sb0's kt-streamed chains
                # consume each pair as it lands
                for k0 in range(0, KT, 2):
                    for v in range(2):
                        nc.sync.dma_start(
                            out=wtq[:, v, k0:k0 + 2, :],
                            in_=wq4[:, v, k0:k0 + 2, :])
                    for v in range(2):
                        nc.sync.dma_start(
                            out=wtk[:, v, k0:k0 + 2, :],
                            in_=wkv4[:, v, k0:k0 + 2, :])

                def wsl_hi(kt0, kt1, n0):  # [128, 2(kt), 256] hi slice
                    if n0 < HPC * HD:
                        return wtq[:, 1, kt0:kt1, n0:n0 + 256]
                    n0 -= HPC * HD
                    return wtk[:, 1, kt0:kt1, n0:n0 + 256]

                def wsl_x(kt, n0):  # [128, 2(lo,hi), 256] cross slice
                    if n0 < HPC * HD:
                        return wtq[:, 0:2, kt, n0:n0 + 256]
                    n0 -= HPC * HD
                    return wtk[:, 0:2, kt, n0:n0 + 256]

                def emit_transposes(sb, rp):
                    for h in range(NR):
                        pt = ptp.tile([128, 1024], bf16, tag="pt")
                        nc.tensor.transpose(
                            pt[:, (h % 4) * 128:(h % 4) * 128 + 128],
                            rp[:, h, :], ident)
                        dst = qT[h] if h < HPC else kT[h - HPC]
                        nc.scalar.copy(
                            out=dst[:, sb * 128:(sb + 1) * 128],
                            in_=pt[:, (h % 4) * 128:(h % 4) * 128 + 128])

                def do_rope(sb, ps):
                    ps3 = ps.rearrange("p (h d) -> p h d", d=128)
                    rp = rpool.tile([128, NR, 128], bf16, tag="rope")
                    t1 = rpool.tile([128, NR, 64], f32, tag="t1")
                    t2 = rpool.tile([128, NR, 64], f32, tag="t2")
                    for h0, nr in ((0, HPC), (HPC, GPC)):  # rope-q, rope-kv
                        ev = ps3[:, h0:h0 + nr, 0:128:2]
                        od = ps3[:, h0:h0 + nr, 1:128:2]
                        cb = cos_t[:, None, sb, :].broadcast_to([128, nr, 64])
                        sn = sin_t[:, None, sb, :].broadcast_to([128, nr, 64])
                        nc.vector.tensor_tensor(
                            out=t1[:, h0:h0 + nr, :], in0=ev, in1=cb,
                            op=ALU.mult)
                        nc.vector.tensor_tensor(
                            out=t2[:, h0:h0 + nr, :], in0=od, in1=sn,
                            op=ALU.mult)
                        nc.vector.tensor_tensor(
                            out=rp[:, h0:h0 + nr, 0:64],
                            in0=t1[:, h0:h0 + nr, :],
                            in1=t2[:, h0:h0 + nr, :], op=ALU.subtract)
                        nc.vector.tensor_tensor(
                            out=t1[:, h0:h0 + nr, :], in0=ev, in1=sn,
                            op=ALU.mult)
                        nc.vector.tensor_tensor(
                            out=t2[:, h0:h0 + nr, :], in0=od, in1=cb,
                            op=ALU.mult)
                        nc.vector.tensor_tensor(
                            out=rp[:, h0:h0 + nr, 64:128],
                            in0=t1[:, h0:h0 + nr, :],
                            in1=t2[:, h0:h0 + nr, :], op=ALU.add)
                    for g in range(GPC):
                        nc.scalar.copy(
                            out=vsb[g][:, sb, :], in_=ps3[:, NR + g, :])
                    return rp

                prev = None  # (sb, rp) whose transposes are deferred one iter
                for sb in range(SB):
                    if sb in (3, 6, 9, 12):
                        c = {3: 0, 6: 1, 9: 2, 12: 3}[sb]
                        for v in range(2):
                            nc.sync.dma_start(
                                out=wos[:, v, :, 512 * c:512 * (c + 1)],
                                in_=wo4[:, v, :, 512 * c:512 * (c + 1)])
                    if sb == 0:
                        xs = xs0
                    else:
                        xs = xpool.tile([128, 2, KT, 128], f8, tag="xs")
                        for v in range(2):
                            nc.sync.dma_start(
                                out=xs[:, v, 0:8, :],
                                in_=xT4[:, v, 0:8, sb * 128:(sb + 1) * 128])
                            nc.sync.dma_start(
                                out=xs[:, v, 8:16, :],
                                in_=xT4[:, v, 8:16, sb * 128:(sb + 1) * 128])
                    ps = pqp.tile([128, EW], f32, tag="ps")
                    # bank-pair-major chains: each bank's two chunk chains run
                    # sequentially; rope-q (banks 0,1) can start on DVE while
                    # the kv chains (bank 2) still run on PE
                    for n0 in (0, 256, 512, 768, 1024, 1280):
                        for j in range(KT // 2):
                            nc.tensor.matmul(
                                ps[:, n0:n0 + 256],
                                xs[:, 0, 2 * j:2 * j + 2, :],
                                wsl_hi(2 * j, 2 * j + 2, n0),
                                start=(j == 0), stop=False, perf_mode=DR,
                                skip_group_check=True)
                        for kt in range(KT):
                            nc.tensor.matmul(
                                ps[:, n0:n0 + 256],
                                xs[:, 0:2, kt, :],
                                wsl_x(kt, n0),
                                start=False, stop=(kt == KT - 1),
                                perf_mode=DR, skip_group_check=True)
                    rp = do_rope(sb, ps)
                    if prev is not None:
                        emit_transposes(*prev)
                    prev = (sb, rp)
                emit_transposes(*prev)

            # ------------ Stage 2+3: attention + out-projection -------------
            with tc.tile_pool(name="pr2", bufs=3) as prpool, \
                 tc.tile_pool(name="att2", bufs=2) as attpool, \
                 tc.tile_pool(name="dn2", bufs=3) as dnpool, \
                 tc.tile_pool(name="o2", bufs=2) as opool, \
                 tc.tile_pool(name="psc", bufs=4, space="PSUM") as pscp, \
                 tc.tile_pool(name="pav", bufs=1, space="PSUM") as pavp, \
                 tc.tile_pool(name="pds", bufs=1, space="PSUM") as pdsp, \
                 tc.tile_pool(name="prs", bufs=1, space="PSUM") as prsp, \
                 tc.tile_pool(name="pou", bufs=1, space="PSUM") as poup:

                pending = []  # (qsb, att_tile, m) out-proj chunks not yet run

                def oproj_chunk(use_psc=False):
                    if not pending:
                        return
                    oq, oatt, m = pending.pop(0)
                    if use_psc:
                        po = pscp.tile([128, 512], f32, tag="sc")
                    else:
                        po = poup.tile([128, 512], f32, tag="po")
                    ms = slice(m * 128, (m + 1) * 128)
                    for half in (0, 1):
                        q0 = half * 256
                        for j in range(HPC // 2):
                            e = 2 * j
                            nc.tensor.matmul(
                                po[:, q0:q0 + 256],
                                wos[:, 1, e:e + 2, ms],
                                oatt[:, e:e + 2, 0, q0:q0 + 256],
                                start=(j == 0), stop=False, perf_mode=DR,
                                skip_group_check=True)
                        for e in range(HPC):
                            nc.tensor.matmul(
                                po[:, q0:q0 + 256],
                                wos[:, 0:2, e, ms],
                                oatt[:, e, 0:2, q0:q0 + 256],
                                start=False, stop=(e == HPC - 1),
                                perf_mode=DR, skip_group_check=True)
                    ot = opool.tile([128, 512], bf16, tag="ot")
                    nc.vector.tensor_copy(out=ot, in_=po[:, 0:512])
                    nc.sync.dma_start(
                        out=outT[m * 128:(m + 1) * 128,
                                 oq * 512:(oq + 1) * 512],
                        in_=ot)

                for qsb in range(QSB):
                    att = attpool.tile([128, HPC, 512], bf16, tag="att")
                    att8 = attpool.tile([128, HPC, 2, 512], f8, tag="att8")
                    maxkt = (qsb + 1) * 4 if causal else SB
                    q0g = qsb * 512
                    for g in range(GPC):
                        pds = pdsp.tile([128, 512], f32, tag="pds")
                        for r in range(NREP):
                            h = g * NREP + r
                            probs = prpool.tile([128, SB, 512], bf16, tag="probs")
                            for t in range(maxkt):
                                ql = max(0, t * 128 - q0g) if causal else 0
                                sc = pscp.tile([128, 512], f32, tag="sc")
                                nc.tensor.matmul(
                                    sc[:, ql:512],
                                    kT[g][:, t * 128:(t + 1) * 128],
                                    qT[h][:, q0g + ql:q0g + 512],
                                    start=True, stop=True)
                                if causal and t * 128 >= q0g:
                                    nc.vector.tensor_tensor(
                                        out=sc[:, ql:ql + 128],
                                        in0=sc[:, ql:ql + 128],
                                        in1=msk, op=ALU.add)
                                nc.scalar.activation(
                                    out=probs[:, t, ql:512],
                                    in_=sc[:, ql:512], func=AF.Exp,
                                    scale=SCALE)
                            oproj_chunk()
                            # AV accumulation (x WS via v scaling)
                            av = pavp.tile([128, 512], f32, tag="av")
                            for t in range(maxkt):
                                ql = max(0, t * 128 - q0g) if causal else 0
                                nc.tensor.matmul(
                                    av[:, ql:512], vsb[g][:, t, :],
                                    probs[:, t, ql:512],
                                    start=(t == 0), stop=(t == maxkt - 1),
                                    skip_group_check=True)
                            # denominators: probs-stationary, sequential chains
                            c0 = r * 4
                            for m in range(4):
                                tmax = min(maxkt, 4 * qsb + m + 1) if causal else SB
                                for t in range(tmax):
                                    nc.tensor.matmul(
                                        pds[:, c0 + m:c0 + m + 1],
                                        probs[:, t, m * 128:(m + 1) * 128],
                                        onec,
                                        start=(t == 0), stop=(t == tmax - 1),
                                        skip_group_check=True)
                            rrs = dnpool.tile([128, 4], f32, tag="rrs")
                            with nc.allow_low_precision(reason="softmax recip"):
                                nc.vector.reciprocal(
                                    out=rrs, in_=pds[:, c0:c0 + 4])
                            # transpose into the dsum bank (safe: no chain is
                            # mid-flight in it here; lazy region zeroing)
                            nc.tensor.transpose(
                                pds[0:4, 16:144], rrs, ident_f)
                            rrc = dnpool.tile([4, 128], bf16, tag="rrc")
                            nc.vector.tensor_copy(out=rrc, in_=pds[0:4, 16:144])
                            rsb = prsp.tile([128, 512], f32, tag="rsb")
                            for m in range(4):
                                nc.tensor.matmul(
                                    rsb[:, m * 128:(m + 1) * 128],
                                    selt[:, m, :], rrc, start=True, stop=True)
                            rsbs = dnpool.tile([128, 512], bf16, tag="rsbs")
                            nc.scalar.copy(out=rsbs, in_=rsb)
                            oproj_chunk()
                            # fused normalize: att = av * (1/den32)
                            nc.vector.tensor_tensor(
                                out=att[:, h, :], in0=av, in1=rsbs,
                                op=ALU.mult)
                            nc.vector.tensor_copy(out=att8[:, h, 0, :],
                                                  in_=att[:, h, :])
                            nc.vector.tensor_tensor(
                                out=att8[:, h, 1, :], in0=att[:, h, :],
                                in1=att8[:, h, 0, :], op=ALU.subtract)
                    pending.extend((qsb, att8, m) for m in range(KT))
                # drain tail, alternating PSUM banks to double-buffer
                i = 0
                while pending:
                    oproj_chunk(use_psc=(i % 2 == 1))
                    i += 1

    nc.compile()
    return nc


def _get_nc(causal: bool):
    if causal not in _compiled:
        _compiled[causal] = _build(causal)
    return _compiled[causal]


def _split8(a):
    import ml_dtypes
    E4 = ml_dtypes.float8_e4m3
    hi = a.astype(E4)
    lo = (a - hi.astype(np.float32)).astype(E4)
    return hi, lo


def kernel(x, freqs_cis, mask, wq, wk, wv, wo):
    import ml_dtypes
    from concourse.bass_utils import run_bass_kernel_spmd
    BF = ml_dtypes.bfloat16

    x = np.asarray(x, dtype=np.float32)
    freqs_cis = np.asarray(freqs_cis, dtype=np.float32)
    mask = np.asarray(mask, dtype=np.float32)
    wq = np.asarray(wq, dtype=np.float32)
    wk = np.asarray(wk, dtype=np.float32)
    wv = np.asarray(wv, dtype=np.float32)
    wo = np.asarray(wo, dtype=np.float32)

    tri = np.tril(np.ones((S, S), dtype=bool))
    causal = bool((mask[tri] == 0.0).all() and (mask[~tri] < -1e30).all())
    if not causal and not (mask == 0.0).all():
        return _numpy_ref(x, freqs_cis, mask, wq, wk, wv, wo)

    nc = _get_nc(causal)

    cos = freqs_cis[:, :, 0] / WS
    sin = freqs_cis[:, :, 1] / WS
    cosS = np.ascontiguousarray(cos.reshape(SB, 128, 64).transpose(1, 0, 2))
    sinS = np.ascontiguousarray(sin.reshape(SB, 128, 64).transpose(1, 0, 2))
    mtile = (np.ascontiguousarray(mask[0:128, 0:128].T) if causal
             else np.zeros((128, 128), dtype=np.float32))
    ones32 = np.full((128, 1), WS, dtype=BF)
    selS = np.zeros((4, 4, 128), dtype=BF)
    for m in range(4):
        selS[m, m, :] = 8.0   # att scaled x8 so its fp8 lo clears subnormals
    selS = selS.reshape(4, 512)

    def pack2(a, b):  # [D, E], [D, E] -> [2, D, E]
        return np.ascontiguousarray(np.stack([a, b], axis=0))

    in_maps = []
    for c in range(8):
        b, i = c // 2, c % 2
        xh, xl = _split8(x[b].T)
        wqh, wql = _split8(wq[1024 * i:1024 * (i + 1), :].T * WS)
        wkvf = np.concatenate(
            [wk[256 * i:256 * (i + 1), :].T,
             wv[256 * i:256 * (i + 1), :].T], axis=1) * WS
        wkh, wkl = _split8(wkvf)
        in_maps.append({
            "xT8": pack2(xh, xl),
            "wq8": pack2(wql, wqh),    # weights: [:,0,:]=lo, [:,1,:]=hi
            "wkv8": pack2(wkl, wkh),
            "wo8": pack2(*reversed(_split8(
                np.ascontiguousarray(
                    wo[:, 1024 * i:1024 * (i + 1)].T) * WS))),
            "cosS": cosS, "sinS": sinS, "mtile": mtile,
            "ones32": ones32, "selS": selS,
        })

    res = run_bass_kernel_spmd(nc, in_maps, core_ids=list(range(8)))
    out = np.empty((B, S, D), dtype=np.float32)
    for b in range(B):
        out[b] = (res.results[2 * b]["outT"].T.astype(np.float32)
                  + res.results[2 * b + 1]["outT"].T.astype(np.float32)) \
            * (1.0 / (WS * 8.0))
    return out


def _numpy_ref(x, freqs_cis, mask, wq, wk, wv, wo):
    xq = (x @ wq.T).reshape(B, S, H, HD)
    xk = (x @ wk.T).reshape(B, S, KV, HD)
    xv = (x @ wv.T).reshape(B, S, KV, HD)

    def rope(xh):
        x2 = xh.reshape(*xh.shape[:-1], HD // 2, 2)
        fc = freqs_cis[None, :, None, :, :]
        real = x2[..., 0] * fc[..., 0] - x2[..., 1] * fc[..., 1]
        imag = x2[..., 0] * fc[..., 1] + x2[..., 1] * fc[..., 0]
        return np.concatenate([real, imag], axis=-1)

    xq, xk = rope(xq), rope(xk)
    q = xq.reshape(B, S, KV, NREP, HD)
    sc = np.einsum('bqgrd,bkgd->bgrqk', q, xk) * SCALE + mask[None, None, None]
    sc = sc - sc.max(axis=-1, keepdims=True)
    p = np.exp(sc)
    p /= p.sum(axis=-1, keepdims=True)
    o = np.einsum('bgrqk,bkgd->bqgrd', p, xv).reshape(B, S, H * HD)
    return (o @ wo.T).astype(np.float32)


# revision 57
# speedup vs baseline: 1.0160x; 1.0160x over previous
"""Trainium2 Bass kernel for nn_Attention (B=4, S=2048, D=2048, H=16, KV=4, HD=128).

Sharding (8 cores): data-parallel over batch (4) x tensor-parallel over
KV-head-group halves (2). Core c handles batch b=c//2 and q-heads
[8*(c%2), 8*(c%2)+8) == kv groups {2*(c%2), 2*(c%2)+1}. Each core produces a
partial output (its heads' contribution through wo); the host sums the two
partials per batch.

v2: Q/K/V projections run as fp8e4 DoubleRow matmuls (0.5 cycles/row, two
128-deep contraction subtiles per instruction) with an error-corrected hi/lo
split of both x and the weights (3 of 4 cross terms; ~2^-8 effective
precision). Weights are pre-scaled x32 on the host so the lo residual clears
e4m3's subnormal floor; the descale folds into cos/sin (q,k) and a 32.0
ones-column in the softmax denominator (v). Attention (scores, exp, AV) and
the output projection run in bf16. Softmax denominators come from
probs-stationary x ones-moving matmuls (out [128q,1], ~1 cycle each instead
of 512); the reciprocal row reaches [dmodel, q] broadcast layout via a tiny
transpose + selector matmuls. Normalization is fused into the PSUM->SBUF
copy of AV. Out-projection chunks are interleaved into the next
q-superblock's attention so the single PSUM bank never stalls the PE.
"""
import numpy as np

B, S, D = 4, 2048, 2048
H, KV, HD = 16, 4, 128
NREP = H // KV
SCALE = float(HD) ** -0.5
WS = 32.0                  # host weight pre-scale (power of 2)

SB = S // 128          # 16 s-blocks
KT = D // 128          # 16 contraction tiles for projections
QSB = S // 512         # 4 q-superblocks
HPC = 8                # q heads per core
GPC = 2                # kv groups per core

_compiled = {}


def _build(causal: bool):
    import concourse.bass as bass  # noqa: F401
    import concourse.tile as tile
    from concourse import bacc, mybir
    from concourse.masks import make_identity

    f32 = mybir.dt.float32
    bf16 = mybir.dt.bfloat16
    f8 = mybir.dt.float8e4
    DR = mybir.MatmulPerfMode.DoubleRow
    AF = mybir.ActivationFunctionType
    ALU = mybir.AluOpType

    nc = bacc.Bacc("TRN2")

    # x hi/lo: [0]=hi, [1]=lo.  weights hi/lo: [0]=LO, [1]=HI (two-major layout)
    xT8 = nc.dram_tensor("xT8", [2, D, S], f8, kind="ExternalInput")
    wq8 = nc.dram_tensor("wq8", [2, D, HPC * HD], f8, kind="ExternalInput")
    wkv8 = nc.dram_tensor("wkv8", [2, D, 2 * GPC * HD], f8, kind="ExternalInput")
    wo8 = nc.dram_tensor("wo8", [2, HPC * HD, D], f8, kind="ExternalInput")
    cosS = nc.dram_tensor("cosS", [128, SB, 64], f32, kind="ExternalInput")
    sinS = nc.dram_tensor("sinS", [128, SB, 64], f32, kind="ExternalInput")
    mtile = nc.dram_tensor("mtile", [128, 128], f32, kind="ExternalInput")
    ones32 = nc.dram_tensor("ones32", [128, 1], bf16, kind="ExternalInput")
    selS = nc.dram_tensor("selS", [4, 4 * 128], bf16, kind="ExternalInput")
    outT = nc.dram_tensor("outT", [D, S], bf16, kind="ExternalOutput")

    xT4 = xT8.rearrange("two (kt p) s -> p two kt s", p=128)
    wq4 = wq8.rearrange("two (kt p) e -> p two kt e", p=128)
    wkv4 = wkv8.rearrange("two (kt p) e -> p two kt e", p=128)
    wo4 = wo8.rearrange("two (h p) d -> p two h d", p=128)

    with tile.TileContext(nc) as tc:
        with tc.tile_pool(name="persist", bufs=1) as persist:
            qT = [persist.tile([128, S], bf16, tag=f"qT{h}", name=f"qT{h}") for h in range(HPC)]
            kT = [persist.tile([128, S], bf16, tag=f"kTg{g}", name=f"kTg{g}") for g in range(GPC)]
            vsb = [persist.tile([128, SB, 128], bf16, tag=f"v{g}", name=f"v{g}") for g in range(GPC)]
            wos = persist.tile([128, 2, HPC, D], f8, tag="wos")
            msk = persist.tile([128, 128], f32, tag="msk")
            nc.sync.dma_start(out=msk, in_=mtile[:, :])
            onec = persist.tile([128, 1], bf16, tag="onec")
            nc.sync.dma_start(out=onec, in_=ones32[:, :])
            selt = persist.tile([4, 4, 128], bf16, tag="selt")
            sel3 = selS.rearrange("k (m d) -> k m d", d=128)
            nc.sync.dma_start(out=selt, in_=sel3[:, :, :])
            ident = persist.tile([128, 128], bf16, tag="ident")
            ident_f = persist.tile([128, 128], f32, tag="identf")
            make_identity(nc, ident_f)
            nc.vector.tensor_copy(out=ident, in_=ident_f)

            cos_t = persist.tile([128, SB, 64], f32, tag="cos")
            sin_t = persist.tile([128, SB, 64], f32, tag="sin")
            nc.sync.dma_start(out=cos_t, in_=cosS[:, :, :])
            nc.sync.dma_start(out=sin_t, in_=sinS[:, :, :])

            # ------------ Stage 1: projections + RoPE + transposes ----------

            # Single merged pass: per s-block compute all 12 projections
            # (8 q | 2 k | 2 v) from one x load. ps columns: q 0:1024,
            # kv 1024:1536 (K heads first). 10 heads get RoPE.
            EW = (HPC + 2 * GPC) * HD  # 1536
            NR = HPC + GPC             # 10 rope heads
            with tc.tile_pool(name="w1", bufs=1) as wpool, \
                 tc.tile_pool(name="xs1", bufs=3) as xpool, \
                 tc.tile_pool(name="rs1", bufs=3) as rpool, \
                 tc.tile_pool(name="pq1", bufs=2, space="PSUM") as pqp, \
                 tc.tile_pool(name="pt1", bufs=2, space="PSUM") as ptp:
                wtq = wpool.tile([128, 2, KT, HPC * HD], f8, tag="wtq")
                wtk = wpool.tile([128, 2, KT, 2 * GPC * HD], f8, tag="wtk")
                # prefetch sb0's x before the weights so compute can stream
                xs0 = xpool.tile([128, 2, KT, 128], f8, tag="xs")
                for v in range(2):
                    nc.sync.dma_start(out=xs0[:, v, 0:8, :],
                                      in_=xT4[:, v, 0:8, 0:128])
                    nc.sync.dma_start(out=xs0[:, v, 8:16, :],
                                      in_=xT4[:, v, 8:16, 0:128])
                # weights interleaved by kt-pair: sb0's kt-streamed chains
                # consume each pair as it lands
                for k0 in range(0, KT, 2):
                    for v in range(2):
                        nc.sync.dma_start(
                            out=wtq[:, v, k0:k0 + 2, :],
                            in_=wq4[:, v, k0:k0 + 2, :])
                    for v in range(2):
                        nc.sync.dma_start(
                            out=wtk[:, v, k0:k0 + 2, :],
                            in_=wkv4[:, v, k0:k0 + 2, :])

                def wsl_hi(kt0, kt1, n0):  # [128, 2(kt), 256] hi slice
                    if n0 < HPC * HD:
                        return wtq[:, 1, kt0:kt1, n0:n0 + 256]
                    n0 -= HPC * HD
                    return wtk[:, 1, kt0:kt1, n0:n0 + 256]

                def wsl_x(kt, n0):  # [128, 2(lo,hi), 256] cross slice
                    if n0 < HPC * HD:
                        return wtq[:, 0:2, kt, n0:n0 + 256]
                    n0 -= HPC * HD
                    return wtk[:, 0:2, kt, n0:n0 + 256]

                def emit_transposes(sb, rp):
                    for h in range(NR):
                        pt = ptp.tile([128, 1024], bf16, tag="pt")
                        nc.tensor.transpose(
                            pt[:, (h % 4) * 128:(h % 4) * 128 + 128],
                            rp[:, h, :], ident)
                        dst = qT[h] if h < HPC else kT[h - HPC]
                        nc.scalar.copy(
                            out=dst[:, sb * 128:(sb + 1) * 128],
                            in_=pt[:, (h % 4) * 128:(h % 4) * 128 + 128])

                def do_rope(sb, ps):
                    ps3 = ps.rearrange("p (h d) -> p h d", d=128)
                    rp = rpool.tile([128, NR, 128], bf16, tag="rope")
                    t1 = rpool.tile([128, NR, 64], f32, tag="t1")
                    t2 = rpool.tile([128, NR, 64], f32, tag="t2")
                    for h0, nr in ((0, HPC), (HPC, GPC)):  # rope-q, rope-kv
                        ev = ps3[:, h0:h0 + nr, 0:128:2]
                        od = ps3[:, h0:h0 + nr, 1:128:2]
                        cb = cos_t[:, None, sb, :].broadcast_to([128, nr, 64])
                        sn = sin_t[:, None, sb, :].broadcast_to([128, nr, 64])
                        nc.vector.tensor_tensor(
                            out=t1[:, h0:h0 + nr, :], in0=ev, in1=cb,
                            op=ALU.mult)
                        nc.vector.tensor_tensor(
                            out=t2[:, h0:h0 + nr, :], in0=od, in1=sn,
                            op=ALU.mult)
                        nc.vector.tensor_tensor(
                            out=rp[:, h0:h0 + nr, 0:64],
                            in0=t1[:, h0:h0 + nr, :],
                            in1=t2[:, h0:h0 + nr, :], op=ALU.subtract)
                        nc.vector.tensor_tensor(
                            out=t1[:, h0:h0 + nr, :], in0=ev, in1=sn,
                            op=ALU.mult)
                        nc.vector.tensor_tensor(
                            out=t2[:, h0:h0 + nr, :], in0=od, in1=cb,
                            op=ALU.mult)
                        nc.vector.tensor_tensor(
                            out=rp[:, h0:h0 + nr, 64:128],
                            in0=t1[:, h0:h0 + nr, :],
                            in1=t2[:, h0:h0 + nr, :], op=ALU.add)
                    for g in range(GPC):
                        nc.scalar.copy(
                            out=vsb[g][:, sb, :], in_=ps3[:, NR + g, :])
                    return rp

                prev = None  # (sb, rp) whose transposes are deferred one iter
                for sb in range(SB):
                    if sb in (3, 6, 9, 12):
                        c = {3: 0, 6: 1, 9: 2, 12: 3}[sb]
                        for v in range(2):
                            nc.sync.dma_start(
                                out=wos[:, v, :, 512 * c:512 * (c + 1)],
                                in_=wo4[:, v, :, 512 * c:512 * (c + 1)])
                    if sb == 0:
                        xs = xs0
                    else:
                        xs = xpool.tile([128, 2, KT, 128], f8, tag="xs")
                        for v in range(2):
                            nc.sync.dma_start(
                                out=xs[:, v, 0:8, :],
                                in_=xT4[:, v, 0:8, sb * 128:(sb + 1) * 128])
                            nc.sync.dma_start(
                                out=xs[:, v, 8:16, :],
                                in_=xT4[:, v, 8:16, sb * 128:(sb + 1) * 128])
                    ps = pqp.tile([128, EW], f32, tag="ps")
                    # bank-pair-major chains: each bank's two chunk chains run
                    # sequentially; rope-q (banks 0,1) can start on DVE while
                    # the kv chains (bank 2) still run on PE
                    for n0 in (0, 256, 512, 768, 1024, 1280):
                        for j in range(KT // 2):
                            nc.tensor.matmul(
                                ps[:, n0:n0 + 256],
                                xs[:, 0, 2 * j:2 * j + 2, :],
                                wsl_hi(2 * j, 2 * j + 2, n0),
                                start=(j == 0), stop=False, perf_mode=DR,
                                skip_group_check=True)
                        for kt in range(KT):
                            nc.tensor.matmul(
                                ps[:, n0:n0 + 256],
                                xs[:, 0:2, kt, :],
                                wsl_x(kt, n0),
                                start=False, stop=(kt == KT - 1),
                                perf_mode=DR, skip_group_check=True)
                    rp = do_rope(sb, ps)
                    if prev is not None:
                        emit_transposes(*prev)
                    prev = (sb, rp)
                emit_transposes(*prev)

            # ------------ Stage 2+3: attention + out-projection -------------
            with tc.tile_pool(name="pr2", bufs=3) as prpool, \
                 tc.tile_pool(name="att2", bufs=3) as attpool, \
                 tc.tile_pool(name="dn2", bufs=3) as dnpool, \
                 tc.tile_pool(name="o2", bufs=4) as opool, \
                 tc.tile_pool(name="psc", bufs=4, space="PSUM") as pscp, \
                 tc.tile_pool(name="pav", bufs=1, space="PSUM") as pavp, \
                 tc.tile_pool(name="pds", bufs=1, space="PSUM") as pdsp, \
                 tc.tile_pool(name="prs", bufs=1, space="PSUM") as prsp, \
                 tc.tile_pool(name="pou", bufs=1, space="PSUM") as poup:

                pending = []  # (qsb, att_tile, m) out-proj chunks not yet run

                def oproj_chunk(use_psc=False):
                    if not pending:
                        return
                    oq, oatt, m = pending.pop(0)
                    if use_psc:
                        po = pscp.tile([128, 512], f32, tag="sc")
                    else:
                        po = poup.tile([128, 512], f32, tag="po")
                    ms = slice(m * 128, (m + 1) * 128)
                    for half in (0, 1):
                        q0 = half * 256
                        for j in range(HPC // 2):
                            e = 2 * j
                            nc.tensor.matmul(
                                po[:, q0:q0 + 256],
                                wos[:, 1, e:e + 2, ms],
                                oatt[:, e:e + 2, 0, q0:q0 + 256],
                                start=(j == 0), stop=False, perf_mode=DR,
                                skip_group_check=True)
                        for e in range(HPC):
                            nc.tensor.matmul(
                                po[:, q0:q0 + 256],
                                wos[:, 0:2, e, ms],
                                oatt[:, e, 0:2, q0:q0 + 256],
                                start=False, stop=(e == HPC - 1),
                                perf_mode=DR, skip_group_check=True)
                    ot = opool.tile([128, 512], bf16, tag="ot")
                    nc.vector.tensor_copy(out=ot, in_=po[:, 0:512])
                    nc.sync.dma_start(
                        out=outT[m * 128:(m + 1) * 128,
                                 oq * 512:(oq + 1) * 512],
                        in_=ot)

                for qsb in range(QSB):
                    att = attpool.tile([128, HPC, 512], bf16, tag="att")
                    att8 = attpool.tile([128, HPC, 2, 512], f8, tag="att8")
                    maxkt = (qsb + 1) * 4 if causal else SB
                    q0g = qsb * 512
                    for g in range(GPC):
                        pds = pdsp.tile([128, 512], f32, tag="pds")
                        for r in range(NREP):
                            h = g * NREP + r
                            probs = prpool.tile([128, SB, 512], bf16, tag="probs")
                            for t in range(maxkt):
                                ql = max(0, t * 128 - q0g) if causal else 0
                                sc = pscp.tile([128, 512], f32, tag="sc")
                                nc.tensor.matmul(
                                    sc[:, ql:512],
                                    kT[g][:, t * 128:(t + 1) * 128],
                                    qT[h][:, q0g + ql:q0g + 512],
                                    start=True, stop=True)
                                if causal and t * 128 >= q0g:
                                    nc.vector.tensor_tensor(
                                        out=sc[:, ql:ql + 128],
                                        in0=sc[:, ql:ql + 128],
                                        in1=msk, op=ALU.add)
                                nc.scalar.activation(
                                    out=probs[:, t, ql:512],
                                    in_=sc[:, ql:512], func=AF.Exp,
                                    scale=SCALE)
                            oproj_chunk()
                            # AV accumulation (x WS via v scaling)
                            av = pavp.tile([128, 512], f32, tag="av")
                            for t in range(maxkt):
                                ql = max(0, t * 128 - q0g) if causal else 0
                                nc.tensor.matmul(
                                    av[:, ql:512], vsb[g][:, t, :],
                                    probs[:, t, ql:512],
                                    start=(t == 0), stop=(t == maxkt - 1),
                                    skip_group_check=True)
                            # denominators: probs-stationary, sequential chains
                            c0 = r * 4
                            for m in range(4):
                                tmax = min(maxkt, 4 * qsb + m + 1) if causal else SB
                                for t in range(tmax):
                                    nc.tensor.matmul(
                                        pds[:, c0 + m:c0 + m + 1],
                                        probs[:, t, m * 128:(m + 1) * 128],
                                        onec,
                                        start=(t == 0), stop=(t == tmax - 1),
                                        skip_group_check=True)
                            rrs = dnpool.tile([128, 4], f32, tag="rrs")
                            with nc.allow_low_precision(reason="softmax recip"):
                                nc.vector.reciprocal(
                                    out=rrs, in_=pds[:, c0:c0 + 4])
                            # transpose into the dsum bank (safe: no chain is
                            # mid-flight in it here; lazy region zeroing)
                            nc.tensor.transpose(
                                pds[0:4, 16:144], rrs, ident_f)
                            rrc = dnpool.tile([4, 128], bf16, tag="rrc")
                            nc.vector.tensor_copy(out=rrc, in_=pds[0:4, 16:144])
                            rsb = prsp.tile([128, 512], f32, tag="rsb")
                            for m in range(4):
                                nc.tensor.matmul(
                                    rsb[:, m * 128:(m + 1) * 128],
                                    selt[:, m, :], rrc, start=True, stop=True)
                            rsbs = dnpool.tile([128, 512], bf16, tag="rsbs")
                            nc.scalar.copy(out=rsbs, in_=rsb)
                            oproj_chunk()
                            # fused normalize: att = av * (1/den32)
                            nc.vector.tensor_tensor(
                                out=att[:, h, :], in0=av, in1=rsbs,
                                op=ALU.mult)
                            nc.vector.tensor_copy(out=att8[:, h, 0, :],
                                                  in_=att[:, h, :])
                            nc.vector.tensor_tensor(
                                out=att8[:, h, 1, :], in0=att[:, h, :],
                                in1=att8[:, h, 0, :], op=ALU.subtract)
                    pending.extend((qsb, att8, m) for m in range(KT))
                # drain tail, alternating PSUM banks to double-buffer
                i = 0
                while pending:
                    oproj_chunk(use_psc=(i % 2 == 1))
                    i += 1

    nc.compile()
    return nc


def _get_nc(causal: bool):
    if causal not in _compiled:
        _compiled[causal] = _build(causal)
    return _compiled[causal]


def _split8(a):
    import ml_dtypes
    E4 = ml_dtypes.float8_e4m3
    hi = a.astype(E4)
    lo = (a - hi.astype(np.float32)).astype(E4)
    return hi, lo


def kernel(x, freqs_cis, mask, wq, wk, wv, wo):
    import ml_dtypes
    from concourse.bass_utils import run_bass_kernel_spmd
    BF = ml_dtypes.bfloat16

    x = np.asarray(x, dtype=np.float32)
    freqs_cis = np.asarray(freqs_cis, dtype=np.float32)
    mask = np.asarray(mask, dtype=np.float32)
    wq = np.asarray(wq, dtype=np.float32)
    wk = np.asarray(wk, dtype=np.float32)
    wv = np.asarray(wv, dtype=np.float32)
    wo = np.asarray(wo, dtype=np.float32)

    tri = np.tril(np.ones((S, S), dtype=bool))
    causal = bool((mask[tri] == 0.0).all() and (mask[~tri] < -1e30).all())
    if not causal and not (mask == 0.0).all():
        return _numpy_ref(x, freqs_cis, mask, wq, wk, wv, wo)

    nc = _get_nc(causal)

    cos = freqs_cis[:, :, 0] / WS
    sin = freqs_cis[:, :, 1] / WS
    cosS = np.ascontiguousarray(cos.reshape(SB, 128, 64).transpose(1, 0, 2))
    sinS = np.ascontiguousarray(sin.reshape(SB, 128, 64).transpose(1, 0, 2))
    mtile = (np.ascontiguousarray(mask[0:128, 0:128].T) if causal
             else np.zeros((128, 128), dtype=np.float32))
    ones32 = np.full((128, 1), WS, dtype=BF)
    selS = np.zeros((4, 4, 128), dtype=BF)
    for m in range(4):
        selS[m, m, :] = 8.0   # att scaled x8 so its fp8 lo clears subnormals
    selS = selS.reshape(4, 512)

    def pack2(a, b):  # [D, E], [D, E] -> [2, D, E]
        return np.ascontiguousarray(np.stack([a, b], axis=0))

    in_maps = []
    for c in range(8):
        b, i = c // 2, c % 2
        xh, xl = _split8(x[b].T)
        wqh, wql = _split8(wq[1024 * i:1024 * (i + 1), :].T * WS)
        wkvf = np.concatenate(
            [wk[256 * i:256 * (i + 1), :].T,
             wv[256 * i:256 * (i + 1), :].T], axis=1) * WS
        wkh, wkl = _split8(wkvf)
        in_maps.append({
            "xT8": pack2(xh, xl),
            "wq8": pack2(wql, wqh),    # weights: [:,0,:]=lo, [:,1,:]=hi
            "wkv8": pack2(wkl, wkh),
            "wo8": pack2(*reversed(_split8(
                np.ascontiguousarray(
                    wo[:, 1024 * i:1024 * (i + 1)].T) * WS))),
            "cosS": cosS, "sinS": sinS, "mtile": mtile,
            "ones32": ones32, "selS": selS,
        })

    res = run_bass_kernel_spmd(nc, in_maps, core_ids=list(range(8)))
    out = np.empty((B, S, D), dtype=np.float32)
    for b in range(B):
        out[b] = (res.results[2 * b]["outT"].T.astype(np.float32)
                  + res.results[2 * b + 1]["outT"].T.astype(np.float32)) \
            * (1.0 / (WS * 8.0))
    return out


def _numpy_ref(x, freqs_cis, mask, wq, wk, wv, wo):
    xq = (x @ wq.T).reshape(B, S, H, HD)
    xk = (x @ wk.T).reshape(B, S, KV, HD)
    xv = (x @ wv.T).reshape(B, S, KV, HD)

    def rope(xh):
        x2 = xh.reshape(*xh.shape[:-1], HD // 2, 2)
        fc = freqs_cis[None, :, None, :, :]
        real = x2[..., 0] * fc[..., 0] - x2[..., 1] * fc[..., 1]
        imag = x2[..., 0] * fc[..., 1] + x2[..., 1] * fc[..., 0]
        return np.concatenate([real, imag], axis=-1)

    xq, xk = rope(xq), rope(xk)
    q = xq.reshape(B, S, KV, NREP, HD)
    sc = np.einsum('bqgrd,bkgd->bgrqk', q, xk) * SCALE + mask[None, None, None]
    sc = sc - sc.max(axis=-1, keepdims=True)
    p = np.exp(sc)
    p /= p.sum(axis=-1, keepdims=True)
    o = np.einsum('bgrqk,bkgd->bqgrd', p, xv).reshape(B, S, H * HD)
    return (o @ wo.T).astype(np.float32)
